# revision 1
# baseline (speedup 1.0000x reference)
"""Trainium2 Bass kernel for nn_MixtureOfExperts_9887014716195.

Strategy: data-parallel over batch (core b <- batch b, B == n_cores == 8).
Each core does: gating softmax + top-2 routing (column layout, cumsum via
PE triangular matmuls), dispatch as PE matmuls with DVE-built one-hot
matrices (eiT = x^T @ disp, giving [d, cap] directly), fp32 expert FFN on
PE, then combine via indirect-DMA row gathers from an expert-output DRAM
table (out[n] = g1[n]*EO[s1[n]] + g2[n]*EO[s2[n]]).

No collectives: gating/expert weights are replicated per core; the loss
reduction is finished on host from tiny per-core stats vectors.
"""

import sys

if "/opt/trn_rl_repo" not in sys.path:
    sys.path.insert(0, "/opt/trn_rl_repo")

import numpy as np

B, N, DIM = 8, 2048, 1024
E, HID = 8, 2048
CAP = 512
EPS = 1e-9
P = 128
NT = N // P          # 16 token tiles
ND = DIM // P        # 8 d tiles
NH = HID // P        # 16 h tiles
NC_ = CAP // P       # 4 cap tiles
LOSS_COEF = 0.01

_CACHE = {}


def build_nc():
    import concourse.bass as bass
    import concourse.mybir as mybir
    import concourse.tile as tile
    from concourse import bacc
    from concourse.masks import make_identity

    f32 = mybir.dt.float32
    i32 = mybir.dt.int32
    u32 = mybir.dt.uint32
    Alu = mybir.AluOpType
    Act = mybir.ActivationFunctionType
    AX = mybir.AxisListType.X

    nc = bacc.Bacc("TRN2", target_bir_lowering=False, debug=False, num_devices=B)

    x_d = nc.dram_tensor("x", [N, DIM], f32, kind="ExternalInput")
    rand_d = nc.dram_tensor("rand", [N], f32, kind="ExternalInput")
    wg_d = nc.dram_tensor("wg", [DIM, E], f32, kind="ExternalInput")
    w1_d = nc.dram_tensor("w1", [E, DIM, HID], f32, kind="ExternalInput")
    w2_d = nc.dram_tensor("w2", [E, HID, DIM], f32, kind="ExternalInput")
    out_d = nc.dram_tensor("out", [N, DIM], f32, kind="ExternalOutput")
    stats_d = nc.dram_tensor("stats", [1, 2 * E], f32, kind="ExternalOutput")
    eo_d = nc.dram_tensor("eo_scratch", [E * CAP, DIM], f32, kind="Internal")

    with tile.TileContext(nc) as tc:
        with (
            tc.tile_pool(name="consts", bufs=1) as cons,
            tc.tile_pool(name="xp", bufs=1) as xp,
            tc.tile_pool(name="pers", bufs=1) as pers,
            tc.tile_pool(name="xt", bufs=3) as xtp,
            tc.tile_pool(name="rt", bufs=4) as rt,
            tc.tile_pool(name="ph2", bufs=1) as ph2,
            tc.tile_pool(name="ph3", bufs=2) as ph3,
            tc.tile_pool(name="ps", bufs=8, space="PSUM") as psp,
        ):
            # ---------------- constants ----------------
            identity = cons.tile([P, P], f32)
            make_identity(nc, identity)
            # strictly-upper triangular ones: su[p, f] = 1 iff p < f
            su = cons.tile([P, P], f32)
            nc.vector.memset(su, 0.0)
            nc.gpsimd.affine_select(
                out=su, in_=su, compare_op=Alu.is_ge, fill=1.0,
                base=0, pattern=[[-1, P]], channel_multiplier=1,
            )
            iota512i = cons.tile([P, CAP], i32)
            nc.gpsimd.iota(iota512i, pattern=[[1, CAP]], base=0, channel_multiplier=0)
            iota512f = cons.tile([P, CAP], f32)
            nc.vector.tensor_copy(iota512f, iota512i)
            iota8i = cons.tile([P, E], i32)
            nc.gpsimd.iota(iota8i, pattern=[[1, E]], base=0, channel_multiplier=0)
            iota8f = cons.tile([P, E], f32)
            nc.vector.tensor_copy(iota8f, iota8i)
            ones_c = cons.tile([P, 1], f32)
            nc.vector.memset(ones_c, 1.0)
            ones_r = cons.tile([1, P], f32)
            nc.vector.memset(ones_r, 1.0)
            cneg1 = cons.tile([P, E], f32)
            nc.vector.memset(cneg1, -1.0)

            # ---------------- inputs to SBUF ----------------
            x_sb = xp.tile([P, NT, DIM], f32)
            for t in range(NT):
                nc.sync.dma_start(out=x_sb[:, t, :], in_=x_d[t * P:(t + 1) * P, :])
            rand_sb = pers.tile([P, NT], f32)
            nc.sync.dma_start(
                out=rand_sb, in_=rand_d[:].rearrange("(t p) -> p t", p=P)
            )
            wg_sb = pers.tile([P, ND, E], f32)
            nc.sync.dma_start(
                out=wg_sb, in_=wg_d[:, :].rearrange("(kt p) e -> p kt e", p=P)
            )

            # ---------------- persistent routing state ----------------
            acc_raw = pers.tile([P, E], f32)
            nc.vector.memset(acc_raw, 0.0)
            acc_m1 = pers.tile([P, E], f32)
            nc.vector.memset(acc_m1, 0.0)
            acc_m1c = pers.tile([P, E], f32)
            nc.vector.memset(acc_m1c, 0.0)
            carry1 = pers.tile([1, NT, E], f32)
            nc.vector.memset(carry1, 0.0)
            carry2 = pers.tile([1, NT, E], f32)
            nc.vector.memset(carry2, 0.0)
            p1_all = pers.tile([P, NT], f32)
            p2_all = pers.tile([P, NT], f32)
            g1f_all = pers.tile([P, NT], f32)
            g2f_all = pers.tile([P, NT], f32)
            g2n_all = pers.tile([P, NT], f32)
            e2x512 = pers.tile([P, NT], f32)
            s1i = pers.tile([P, NT], i32)
            s2i = pers.tile([P, NT], i32)
            p1e_sb = pers.tile([P, NT, E], f32)
            p2e_sb = pers.tile([P, NT, E], f32)
            c2sb = pers.tile([P, NT, E], f32)
            m2sb = pers.tile([P, NT, E], f32)
            m1cnt_sb = pers.tile([1, E], f32)
            m1bc = pers.tile([P, E], f32)
            stats_sb = pers.tile([1, 2 * E], f32)

            # ================ Phase 1A: routing, per token tile ================
            for t in range(NT):
                # gating logits: out[tok, e] += xT_km.T @ wg_km
                lg = psp.tile([P, E], f32, tag="ps")
                for km in range(ND):
                    tp = psp.tile([P, P], f32, tag="ps")
                    nc.tensor.transpose(
                        tp, x_sb[:, t, km * P:(km + 1) * P], identity
                    )
                    xt = xtp.tile([P, P], f32, tag="xt")
                    nc.scalar.copy(xt, tp)
                    nc.tensor.matmul(
                        lg, lhsT=xt, rhs=wg_sb[:, km, :],
                        start=(km == 0), stop=(km == ND - 1),
                    )
                # softmax over e (free dim)
                negm = rt.tile([P, 1], f32, tag="r1")
                nc.vector.tensor_reduce(negm, lg, axis=AX, op=Alu.max, negate=True)
                raws = rt.tile([P, E], f32, tag="r8")
                nc.scalar.activation(raws, lg, Act.Exp, bias=negm, scale=1.0)
                ssum = rt.tile([P, 1], f32, tag="r1")
                nc.vector.tensor_reduce(ssum, raws, axis=AX, op=Alu.add)
                rsum = rt.tile([P, 1], f32, tag="r1")
                nc.vector.reciprocal(rsum, ssum)
                raw = rt.tile([P, E], f32, tag="r8")
                nc.vector.tensor_scalar_mul(raw, raws, rsum)
                nc.vector.tensor_add(acc_raw, acc_raw, raw)

                # top-2
                mx = rt.tile([P, E], f32, tag="r8")
                nc.vector.max(out=mx, in_=raw)
                idx = rt.tile([P, E], u32, tag="ri")
                nc.vector.max_index(out=idx, in_max=mx, in_values=raw)
                e12f = rt.tile([P, 2], f32, tag="r2")
                nc.vector.tensor_copy(e12f, idx[:, 0:2])

                mask1 = rt.tile([P, E], f32, tag="r8")
                nc.vector.tensor_scalar(
                    mask1, iota8f, e12f[:, 0:1], None, op0=Alu.is_equal
                )
                nc.vector.tensor_add(acc_m1, acc_m1, mask1)
                mask2e = rt.tile([P, E], f32, tag="r8")
                nc.vector.tensor_scalar(
                    mask2e, iota8f, e12f[:, 1:2], None, op0=Alu.is_equal
                )

                # normalized gates
                den = rt.tile([P, 1], f32, tag="r1")
                nc.vector.tensor_tensor(den, mx[:, 0:1], mx[:, 1:2], op=Alu.add)
                nc.vector.tensor_scalar(den, den, float(EPS), None, op0=Alu.add)
                rden = rt.tile([P, 1], f32, tag="r1")
                nc.vector.reciprocal(rden, den)
                g1n = rt.tile([P, 1], f32, tag="r1")
                nc.vector.tensor_scalar_mul(g1n, mx[:, 0:1], rden)
                nc.vector.tensor_scalar_mul(g2n_all[:, t:t + 1], mx[:, 1:2], rden)

                # random second-expert keep: rand < g2 / 0.2
                thr = rt.tile([P, 1], f32, tag="r1")
                nc.vector.tensor_scalar_mul(thr, g2n_all[:, t:t + 1], 5.0)
                keep = rt.tile([P, 1], f32, tag="r1")
                nc.vector.tensor_tensor(
                    keep, thr, rand_sb[:, t:t + 1], op=Alu.is_gt
                )
                mask2 = rt.tile([P, E], f32, tag="r8")
                nc.vector.tensor_scalar_mul(mask2, mask2e, keep)
                nc.vector.tensor_copy(m2sb[:, t, :], mask2)

                # --- cumsum over tokens for expert-1 mask ---
                cs1 = psp.tile([1, E], f32, tag="ps")
                nc.tensor.matmul(cs1, lhsT=ones_c, rhs=mask1, start=True, stop=True)
                if t + 1 < NT:
                    nc.vector.tensor_tensor(
                        carry1[:, t + 1, :], carry1[:, t, :], cs1[0:1, :], op=Alu.add
                    )
                pos1 = psp.tile([P, E], f32, tag="ps")
                nc.tensor.matmul(pos1, lhsT=su, rhs=mask1, start=True, stop=False)
                nc.tensor.matmul(
                    pos1, lhsT=ones_r, rhs=carry1[:, t, :], start=False, stop=True
                )

                # capacity filter for expert-1
                mask1c = rt.tile([P, E], f32, tag="r8")
                valid1 = rt.tile([P, 1], f32, tag="r1")
                nc.vector.scalar_tensor_tensor(
                    out=mask1c, in0=pos1, scalar=float(CAP), in1=mask1,
                    op0=Alu.is_lt, op1=Alu.mult, accum_out=valid1,
                )
                nc.vector.tensor_add(acc_m1c, acc_m1c, mask1c)
                junk = rt.tile([P, E], f32, tag="r8")
                nc.vector.scalar_tensor_tensor(
                    out=junk, in0=pos1, scalar=0.0, in1=mask1,
                    op0=Alu.add, op1=Alu.mult, accum_out=p1_all[:, t:t + 1],
                )
                p1p1 = rt.tile([P, 1], f32, tag="r1")
                nc.vector.tensor_scalar(
                    p1p1, p1_all[:, t:t + 1], 1.0, None, op0=Alu.add
                )
                nc.vector.scalar_tensor_tensor(
                    out=p1e_sb[:, t, :], in0=mask1, scalar=p1p1, in1=cneg1,
                    op0=Alu.mult, op1=Alu.add,
                )
                nc.vector.tensor_tensor(
                    g1f_all[:, t:t + 1], g1n, valid1, op=Alu.mult
                )
                # s1 = (e1*512 + p1) * valid1
                b5 = rt.tile([P, 1], f32, tag="r1")
                nc.vector.tensor_scalar_mul(b5, e12f[:, 0:1], float(CAP))
                a5 = rt.tile([P, 1], f32, tag="r1")
                nc.vector.tensor_tensor(a5, p1_all[:, t:t + 1], valid1, op=Alu.mult)
                s1f = rt.tile([P, 1], f32, tag="r1")
                nc.vector.scalar_tensor_tensor(
                    out=s1f, in0=b5, scalar=valid1, in1=a5,
                    op0=Alu.mult, op1=Alu.add,
                )
                nc.vector.tensor_copy(s1i[:, t:t + 1], s1f)
                nc.vector.tensor_scalar_mul(e2x512[:, t:t + 1], e12f[:, 1:2], float(CAP))

                # --- cumsum over tokens for expert-2 mask (pre m1cnt offset) ---
                cs2 = psp.tile([1, E], f32, tag="ps")
                nc.tensor.matmul(cs2, lhsT=ones_c, rhs=mask2, start=True, stop=True)
                if t + 1 < NT:
                    nc.vector.tensor_tensor(
                        carry2[:, t + 1, :], carry2[:, t, :], cs2[0:1, :], op=Alu.add
                    )
                pos2 = psp.tile([P, E], f32, tag="ps")
                nc.tensor.matmul(pos2, lhsT=su, rhs=mask2, start=True, stop=False)
                nc.tensor.matmul(
                    pos2, lhsT=ones_r, rhs=carry2[:, t, :], start=False, stop=True
                )
                nc.scalar.copy(c2sb[:, t, :], pos2)

            # mask_1 count per expert (post capacity) and its broadcast
            m1cnt_ps = psp.tile([1, E], f32, tag="ps")
            nc.tensor.matmul(m1cnt_ps, lhsT=ones_c, rhs=acc_m1c, start=True, stop=True)
            nc.scalar.copy(m1cnt_sb, m1cnt_ps)
            m1bc_ps = psp.tile([P, E], f32, tag="ps")
            nc.tensor.matmul(m1bc_ps, lhsT=ones_r, rhs=m1cnt_sb, start=True, stop=True)
            nc.scalar.copy(m1bc, m1bc_ps)

            # ================ Phase 1B: finish expert-2 routing ================
            for t in range(NT):
                pos2f = rt.tile([P, E], f32, tag="r8")
                nc.vector.tensor_tensor(pos2f, c2sb[:, t, :], m1bc, op=Alu.add)
                mask2c = rt.tile([P, E], f32, tag="r8")
                valid2 = rt.tile([P, 1], f32, tag="r1")
                nc.vector.scalar_tensor_tensor(
                    out=mask2c, in0=pos2f, scalar=float(CAP), in1=m2sb[:, t, :],
                    op0=Alu.is_lt, op1=Alu.mult, accum_out=valid2,
                )
                junk2 = rt.tile([P, E], f32, tag="r8")
                nc.vector.scalar_tensor_tensor(
                    out=junk2, in0=pos2f, scalar=0.0, in1=m2sb[:, t, :],
                    op0=Alu.add, op1=Alu.mult, accum_out=p2_all[:, t:t + 1],
                )
                p2p1 = rt.tile([P, 1], f32, tag="r1")
                nc.vector.tensor_scalar(
                    p2p1, p2_all[:, t:t + 1], 1.0, None, op0=Alu.add
                )
                nc.vector.scalar_tensor_tensor(
                    out=p2e_sb[:, t, :], in0=m2sb[:, t, :], scalar=p2p1, in1=cneg1,
                    op0=Alu.mult, op1=Alu.add,
                )
                nc.vector.tensor_tensor(
                    g2f_all[:, t:t + 1], g2n_all[:, t:t + 1], valid2, op=Alu.mult
                )
                a6 = rt.tile([P, 1], f32, tag="r1")
                nc.vector.tensor_tensor(a6, p2_all[:, t:t + 1], valid2, op=Alu.mult)
                s2f = rt.tile([P, 1], f32, tag="r1")
                nc.vector.scalar_tensor_tensor(
                    out=s2f, in0=e2x512[:, t:t + 1], scalar=valid2, in1=a6,
                    op0=Alu.mult, op1=Alu.add,
                )
                nc.vector.tensor_copy(s2i[:, t:t + 1], s2f)

            # loss stats: col-sums of raw gates and pre-capacity mask_1
            st1 = psp.tile([1, E], f32, tag="ps")
            nc.tensor.matmul(st1, lhsT=ones_c, rhs=acc_raw, start=True, stop=True)
            nc.scalar.copy(stats_sb[:, 0:E], st1)
            st2 = psp.tile([1, E], f32, tag="ps")
            nc.tensor.matmul(st2, lhsT=ones_c, rhs=acc_m1, start=True, stop=True)
            nc.scalar.copy(stats_sb[:, E:2 * E], st2)
            nc.sync.dma_start(out=stats_d[:, :], in_=stats_sb)

            # ================ Phase 2: per-expert dispatch + FFN ================
            for e in range(E):
                # dispatch one-hots + eiT = x^T @ disp  -> [DIM, CAP]
                ei_ps = []
                for m in range(ND):
                    ei_ps.append(psp.tile([P, CAP], f32, tag="ps", name=f"eips{e}_{m}"))
                for t in range(NT):
                    disp = ph2.tile([P, CAP], f32, tag="disp", bufs=3)
                    nc.vector.tensor_scalar(
                        disp, iota512f, p1e_sb[:, t, e:e + 1], None, op0=Alu.is_equal
                    )
                    nc.vector.scalar_tensor_tensor(
                        out=disp, in0=iota512f, scalar=p2e_sb[:, t, e:e + 1],
                        in1=disp, op0=Alu.is_equal, op1=Alu.add,
                    )
                    for m in range(ND):
                        nc.tensor.matmul(
                            ei_ps[m], lhsT=x_sb[:, t, m * P:(m + 1) * P], rhs=disp,
                            start=(t == 0), stop=(t == NT - 1),
                        )
                eiT_sb = ph2.tile([P, ND, CAP], f32, tag="eit", bufs=1)
                for m in range(ND):
                    nc.scalar.copy(eiT_sb[:, m, :], ei_ps[m])

                # FFN1 + gelu: hT[h, cap]
                hT_sb = ph2.tile([P, NH, CAP], f32, tag="ht", bufs=1)
                for hm in range(NH):
                    w1s = ph2.tile([P, ND, P], f32, tag="w1s", bufs=2)
                    nc.sync.dma_start(
                        out=w1s,
                        in_=w1_d[e, :, hm * P:(hm + 1) * P].rearrange(
                            "(kt p) h -> p kt h", p=P
                        ),
                    )
                    h_ps = psp.tile([P, CAP], f32, tag="ps", name=f"hps{e}_{hm}")
                    for km in range(ND):
                        nc.tensor.matmul(
                            h_ps, lhsT=w1s[:, km, :], rhs=eiT_sb[:, km, :],
                            start=(km == 0), stop=(km == ND - 1),
                        )
                    nc.scalar.activation(hT_sb[:, hm, :], h_ps, Act.Gelu)

                # FFN2: EO[cap, d] accumulated over h tiles
                eo_ps = []
                for j in range(NC_ * 2):
                    eo_ps.append(psp.tile([P, CAP], f32, tag="ps", name=f"eops{e}_{j}"))
                for kh in range(NH):
                    w2s = ph2.tile([P, DIM], f32, tag="w2s", bufs=2)
                    nc.sync.dma_start(out=w2s, in_=w2_d[e, kh * P:(kh + 1) * P, :])
                    for cm in range(NC_):
                        for dn in range(2):
                            nc.tensor.matmul(
                                eo_ps[cm * 2 + dn],
                                lhsT=hT_sb[:, kh, cm * P:(cm + 1) * P],
                                rhs=w2s[:, dn * CAP:(dn + 1) * CAP],
                                start=(kh == 0), stop=(kh == NH - 1),
                            )
                eo_sb = ph2.tile([P, NC_ * 2, CAP], f32, tag="eo", bufs=1)
                for cm in range(NC_):
                    for dn in range(2):
                        nc.scalar.copy(eo_sb[:, cm * 2 + dn, :], eo_ps[cm * 2 + dn])
                        nc.sync.dma_start(
                            out=eo_d[
                                e * CAP + cm * P: e * CAP + (cm + 1) * P,
                                dn * CAP:(dn + 1) * CAP,
                            ],
                            in_=eo_sb[:, cm * 2 + dn, :],
                        )

            # ================ Phase 3: combine via row gathers ================
            import concourse.bass as bass_mod

            for t in range(NT):
                gA = ph3.tile([P, DIM], f32, tag="gA", bufs=2)
                nc.gpsimd.indirect_dma_start(
                    out=gA, out_offset=None, in_=eo_d[:, :],
                    in_offset=bass_mod.IndirectOffsetOnAxis(
                        ap=s1i[:, t:t + 1], axis=0
                    ),
                )
                gB = ph3.tile([P, DIM], f32, tag="gB", bufs=2)
                nc.gpsimd.indirect_dma_start(
                    out=gB, out_offset=None, in_=eo_d[:, :],
                    in_offset=bass_mod.IndirectOffsetOnAxis(
                        ap=s2i[:, t:t + 1], axis=0
                    ),
                )
                nc.vector.tensor_scalar_mul(gA, gA, g1f_all[:, t:t + 1])
                outt = ph3.tile([P, DIM], f32, tag="outt", bufs=2)
                nc.vector.scalar_tensor_tensor(
                    out=outt, in0=gB, scalar=g2f_all[:, t:t + 1], in1=gA,
                    op0=Alu.mult, op1=Alu.add,
                )
                nc.sync.dma_start(out=out_d[t * P:(t + 1) * P, :], in_=outt)

    nc.compile()
    return nc


def _get_nc():
    if "nc" not in _CACHE:
        _CACHE["nc"] = build_nc()
    return _CACHE["nc"]


def kernel(x, w_gating, w1, w2, rand_probs):
    from concourse.bass_utils import run_bass_kernel_spmd

    nc = _get_nc()
    x = np.ascontiguousarray(x, dtype=np.float32)
    rand_probs = np.ascontiguousarray(rand_probs, dtype=np.float32)
    w_gating = np.ascontiguousarray(w_gating, dtype=np.float32)
    w1 = np.ascontiguousarray(w1, dtype=np.float32)
    w2 = np.ascontiguousarray(w2, dtype=np.float32)

    in_maps = [
        {"x": x[b], "rand": rand_probs[b], "wg": w_gating, "w1": w1, "w2": w2}
        for b in range(B)
    ]
    res = run_bass_kernel_spmd(nc, in_maps, core_ids=list(range(B)))
    out = np.stack([res.results[b]["out"] for b in range(B)])
    stats = np.stack([res.results[b]["stats"][0] for b in range(B)])  # [B, 16]
    proxy_mean = stats[:, 0:E] / np.float32(N)
    density = stats[:, E:2 * E] / np.float32(N)
    loss = np.float32(np.mean(proxy_mean * density) * (E * E) * LOSS_COEF)
    return out, loss


# revision 4
# speedup vs baseline: 2.1179x; 2.1179x over previous
"""Trainium2 Bass kernel for nn_MixtureOfExperts_9887014716195.

Strategy: data-parallel over batch (core b <- batch b, B == n_cores == 8).
Each core does: gating softmax + top-2 routing (column layout, cumsum via
PE triangular matmuls), dispatch as PE matmuls with DVE-built one-hot
matrices (eiT = x^T @ disp, giving [d, cap] directly), fp32 expert FFN on
PE, then combine via indirect-DMA row gathers from an expert-output DRAM
table (out[n] = g1[n]*EO[s1[n]] + g2[n]*EO[s2[n]]).

No collectives: gating/expert weights are replicated per core; the loss
reduction is finished on host from tiny per-core stats vectors.
"""

import sys

if "/opt/trn_rl_repo" not in sys.path:
    sys.path.insert(0, "/opt/trn_rl_repo")

import numpy as np

B, N, DIM = 8, 2048, 1024
E, HID = 8, 2048
CAP = 512
EPS = 1e-9
P = 128
NT = N // P          # 16 token tiles
ND = DIM // P        # 8 d tiles
NH = HID // P        # 16 h tiles
NC_ = CAP // P       # 4 cap tiles
LOSS_COEF = 0.01

_CACHE = {}


def build_nc():
    import concourse.bass as bass
    import concourse.mybir as mybir
    import concourse.tile as tile
    from concourse import bacc
    from concourse.masks import make_identity

    f32 = mybir.dt.float32
    f32r = mybir.dt.float32r
    i32 = mybir.dt.int32
    u32 = mybir.dt.uint32
    Alu = mybir.AluOpType
    Act = mybir.ActivationFunctionType
    AX = mybir.AxisListType.X

    nc = bacc.Bacc("TRN2", target_bir_lowering=False, debug=False, num_devices=B)

    x_d = nc.dram_tensor("x", [N, DIM], f32, kind="ExternalInput")
    xr_d = nc.dram_tensor("xr", [N, DIM], f32r, kind="ExternalInput")
    rand_d = nc.dram_tensor("rand", [N], f32, kind="ExternalInput")
    wg_d = nc.dram_tensor("wg", [DIM, E], f32, kind="ExternalInput")
    w1_d = nc.dram_tensor("w1", [E, DIM, HID], f32r, kind="ExternalInput")
    w2_d = nc.dram_tensor("w2", [E, HID, DIM], f32r, kind="ExternalInput")
    out_d = nc.dram_tensor("out", [N, DIM], f32, kind="ExternalOutput")
    stats_d = nc.dram_tensor("stats", [1, 2 * E], f32, kind="ExternalOutput")
    eo_d = nc.dram_tensor("eo_scratch", [E * CAP, DIM], f32, kind="Internal")

    with tile.TileContext(nc) as tc:
        with (
            tc.tile_pool(name="consts", bufs=1) as cons,
            tc.tile_pool(name="xp", bufs=1) as xp,
            tc.tile_pool(name="pers", bufs=1) as pers,
            tc.tile_pool(name="xt", bufs=3) as xtp,
            tc.tile_pool(name="rt", bufs=4) as rt,
            tc.tile_pool(name="ph2", bufs=1) as ph2,
            tc.tile_pool(name="ph3", bufs=2) as ph3,
            tc.tile_pool(name="ps", bufs=8, space="PSUM") as psp,
        ):
            # ---------------- constants ----------------
            identity = cons.tile([P, P], f32)
            make_identity(nc, identity)
            # strictly-upper triangular ones: su[p, f] = 1 iff p < f
            su = cons.tile([P, P], f32)
            nc.vector.memset(su, 0.0)
            nc.gpsimd.affine_select(
                out=su, in_=su, compare_op=Alu.is_ge, fill=1.0,
                base=0, pattern=[[-1, P]], channel_multiplier=1,
            )
            iota512i = cons.tile([P, CAP], i32)
            nc.gpsimd.iota(iota512i, pattern=[[1, CAP]], base=0, channel_multiplier=0)
            iota512f = cons.tile([P, CAP], f32)
            nc.vector.tensor_copy(iota512f, iota512i)
            iota8i = cons.tile([P, E], i32)
            nc.gpsimd.iota(iota8i, pattern=[[1, E]], base=0, channel_multiplier=0)
            iota8f = cons.tile([P, E], f32)
            nc.vector.tensor_copy(iota8f, iota8i)
            ones_c = cons.tile([P, 1], f32)
            nc.vector.memset(ones_c, 1.0)
            ones_r = cons.tile([1, P], f32)
            nc.vector.memset(ones_r, 1.0)
            cneg1 = cons.tile([P, E], f32)
            nc.vector.memset(cneg1, -1.0)

            # ---------------- inputs to SBUF ----------------
            x_sb = xp.tile([P, NT, DIM], f32r)
            for t in range(NT):
                nc.sync.dma_start(out=x_sb[:, t, :], in_=xr_d[t * P:(t + 1) * P, :])
            rand_sb = pers.tile([P, NT], f32)
            nc.sync.dma_start(
                out=rand_sb, in_=rand_d[:].rearrange("(t p) -> p t", p=P)
            )
            wg_sb = pers.tile([P, ND, E], f32)
            nc.sync.dma_start(
                out=wg_sb, in_=wg_d[:, :].rearrange("(kt p) e -> p kt e", p=P)
            )

            # ---------------- persistent routing state ----------------
            acc_raw = pers.tile([P, E], f32)
            nc.vector.memset(acc_raw, 0.0)
            acc_m1 = pers.tile([P, E], f32)
            nc.vector.memset(acc_m1, 0.0)
            acc_m1c = pers.tile([P, E], f32)
            nc.vector.memset(acc_m1c, 0.0)
            carry1 = pers.tile([1, NT, E], f32)
            nc.vector.memset(carry1, 0.0)
            carry2 = pers.tile([1, NT, E], f32)
            nc.vector.memset(carry2, 0.0)
            p1_all = pers.tile([P, NT], f32)
            p2_all = pers.tile([P, NT], f32)
            g1f_all = pers.tile([P, NT], f32)
            g2f_all = pers.tile([P, NT], f32)
            g2n_all = pers.tile([P, NT], f32)
            e2x512 = pers.tile([P, NT], f32)
            s1i = pers.tile([P, NT], i32)
            s2i = pers.tile([P, NT], i32)
            p1e_sb = pers.tile([P, NT, E], f32)
            p2e_sb = pers.tile([P, NT, E], f32)
            c2sb = pers.tile([P, NT, E], f32)
            m2sb = pers.tile([P, NT, E], f32)
            m1cnt_sb = pers.tile([1, E], f32)
            m1bc = pers.tile([P, E], f32)
            stats_sb = pers.tile([1, 2 * E], f32)

            # ================ Phase 1A: routing, per token tile ================
            for t in range(NT):
                # gating logits: out[tok, e] += xT_km.T @ wg_km
                lg = psp.tile([P, E], f32, tag="ps")
                xin = xtp.tile([P, DIM], f32, tag="xin", bufs=3)
                nc.sync.dma_start(out=xin, in_=x_d[t * P:(t + 1) * P, :])
                for km in range(ND):
                    tp = psp.tile([P, P], f32, tag="ps")
                    nc.tensor.transpose(
                        tp, xin[:, km * P:(km + 1) * P], identity
                    )
                    xt = xtp.tile([P, P], f32, tag="xt")
                    nc.scalar.copy(xt, tp)
                    nc.tensor.matmul(
                        lg, lhsT=xt, rhs=wg_sb[:, km, :],
                        start=(km == 0), stop=(km == ND - 1),
                    )
                # softmax over e (free dim)
                negm = rt.tile([P, 1], f32, tag="r1")
                nc.vector.tensor_reduce(negm, lg, axis=AX, op=Alu.max, negate=True)
                raws = rt.tile([P, E], f32, tag="r8")
                nc.scalar.activation(raws, lg, Act.Exp, bias=negm, scale=1.0)
                ssum = rt.tile([P, 1], f32, tag="r1")
                nc.vector.tensor_reduce(ssum, raws, axis=AX, op=Alu.add)
                rsum = rt.tile([P, 1], f32, tag="r1")
                nc.vector.reciprocal(rsum, ssum)
                raw = rt.tile([P, E], f32, tag="r8")
                nc.vector.tensor_scalar_mul(raw, raws, rsum)
                nc.vector.tensor_add(acc_raw, acc_raw, raw)

                # top-2
                mx = rt.tile([P, E], f32, tag="r8")
                nc.vector.max(out=mx, in_=raw)
                idx = rt.tile([P, E], u32, tag="ri")
                nc.vector.max_index(out=idx, in_max=mx, in_values=raw)
                e12f = rt.tile([P, 2], f32, tag="r2")
                nc.vector.tensor_copy(e12f, idx[:, 0:2])

                mask1 = rt.tile([P, E], f32, tag="r8")
                nc.vector.tensor_scalar(
                    mask1, iota8f, e12f[:, 0:1], None, op0=Alu.is_equal
                )
                nc.vector.tensor_add(acc_m1, acc_m1, mask1)
                mask2e = rt.tile([P, E], f32, tag="r8")
                nc.vector.tensor_scalar(
                    mask2e, iota8f, e12f[:, 1:2], None, op0=Alu.is_equal
                )

                # normalized gates
                den = rt.tile([P, 1], f32, tag="r1")
                nc.vector.tensor_tensor(den, mx[:, 0:1], mx[:, 1:2], op=Alu.add)
                nc.vector.tensor_scalar(den, den, float(EPS), None, op0=Alu.add)
                rden = rt.tile([P, 1], f32, tag="r1")
                nc.vector.reciprocal(rden, den)
                g1n = rt.tile([P, 1], f32, tag="r1")
                nc.vector.tensor_scalar_mul(g1n, mx[:, 0:1], rden)
                nc.vector.tensor_scalar_mul(g2n_all[:, t:t + 1], mx[:, 1:2], rden)

                # random second-expert keep: rand < g2 / 0.2
                thr = rt.tile([P, 1], f32, tag="r1")
                nc.vector.tensor_scalar_mul(thr, g2n_all[:, t:t + 1], 5.0)
                keep = rt.tile([P, 1], f32, tag="r1")
                nc.vector.tensor_tensor(
                    keep, thr, rand_sb[:, t:t + 1], op=Alu.is_gt
                )
                mask2 = rt.tile([P, E], f32, tag="r8")
                nc.vector.tensor_scalar_mul(mask2, mask2e, keep)
                nc.vector.tensor_copy(m2sb[:, t, :], mask2)

                # --- cumsum over tokens for expert-1 mask ---
                cs1 = psp.tile([1, E], f32, tag="ps")
                nc.tensor.matmul(cs1, lhsT=ones_c, rhs=mask1, start=True, stop=True)
                if t + 1 < NT:
                    nc.vector.tensor_tensor(
                        carry1[:, t + 1, :], carry1[:, t, :], cs1[0:1, :], op=Alu.add
                    )
                pos1 = psp.tile([P, E], f32, tag="ps")
                nc.tensor.matmul(pos1, lhsT=su, rhs=mask1, start=True, stop=False)
                nc.tensor.matmul(
                    pos1, lhsT=ones_r, rhs=carry1[:, t, :], start=False, stop=True
                )

                # capacity filter for expert-1
                mask1c = rt.tile([P, E], f32, tag="r8")
                valid1 = rt.tile([P, 1], f32, tag="r1")
                nc.vector.scalar_tensor_tensor(
                    out=mask1c, in0=pos1, scalar=float(CAP), in1=mask1,
                    op0=Alu.is_lt, op1=Alu.mult, accum_out=valid1,
                )
                nc.vector.tensor_add(acc_m1c, acc_m1c, mask1c)
                junk = rt.tile([P, E], f32, tag="r8")
                nc.vector.scalar_tensor_tensor(
                    out=junk, in0=pos1, scalar=0.0, in1=mask1,
                    op0=Alu.add, op1=Alu.mult, accum_out=p1_all[:, t:t + 1],
                )
                p1p1 = rt.tile([P, 1], f32, tag="r1")
                nc.vector.tensor_scalar(
                    p1p1, p1_all[:, t:t + 1], 1.0, None, op0=Alu.add
                )
                nc.vector.scalar_tensor_tensor(
                    out=p1e_sb[:, t, :], in0=mask1, scalar=p1p1, in1=cneg1,
                    op0=Alu.mult, op1=Alu.add,
                )
                nc.vector.tensor_tensor(
                    g1f_all[:, t:t + 1], g1n, valid1, op=Alu.mult
                )
                # s1 = (e1*512 + p1) * valid1
                b5 = rt.tile([P, 1], f32, tag="r1")
                nc.vector.tensor_scalar_mul(b5, e12f[:, 0:1], float(CAP))
                a5 = rt.tile([P, 1], f32, tag="r1")
                nc.vector.tensor_tensor(a5, p1_all[:, t:t + 1], valid1, op=Alu.mult)
                s1f = rt.tile([P, 1], f32, tag="r1")
                nc.vector.scalar_tensor_tensor(
                    out=s1f, in0=b5, scalar=valid1, in1=a5,
                    op0=Alu.mult, op1=Alu.add,
                )
                nc.vector.tensor_copy(s1i[:, t:t + 1], s1f)
                nc.vector.tensor_scalar_mul(e2x512[:, t:t + 1], e12f[:, 1:2], float(CAP))

                # --- cumsum over tokens for expert-2 mask (pre m1cnt offset) ---
                cs2 = psp.tile([1, E], f32, tag="ps")
                nc.tensor.matmul(cs2, lhsT=ones_c, rhs=mask2, start=True, stop=True)
                if t + 1 < NT:
                    nc.vector.tensor_tensor(
                        carry2[:, t + 1, :], carry2[:, t, :], cs2[0:1, :], op=Alu.add
                    )
                pos2 = psp.tile([P, E], f32, tag="ps")
                nc.tensor.matmul(pos2, lhsT=su, rhs=mask2, start=True, stop=False)
                nc.tensor.matmul(
                    pos2, lhsT=ones_r, rhs=carry2[:, t, :], start=False, stop=True
                )
                nc.scalar.copy(c2sb[:, t, :], pos2)

            # mask_1 count per expert (post capacity) and its broadcast
            m1cnt_ps = psp.tile([1, E], f32, tag="ps")
            nc.tensor.matmul(m1cnt_ps, lhsT=ones_c, rhs=acc_m1c, start=True, stop=True)
            nc.scalar.copy(m1cnt_sb, m1cnt_ps)
            m1bc_ps = psp.tile([P, E], f32, tag="ps")
            nc.tensor.matmul(m1bc_ps, lhsT=ones_r, rhs=m1cnt_sb, start=True, stop=True)
            nc.scalar.copy(m1bc, m1bc_ps)

            # ================ Phase 1B: finish expert-2 routing ================
            for t in range(NT):
                pos2f = rt.tile([P, E], f32, tag="r8")
                nc.vector.tensor_tensor(pos2f, c2sb[:, t, :], m1bc, op=Alu.add)
                mask2c = rt.tile([P, E], f32, tag="r8")
                valid2 = rt.tile([P, 1], f32, tag="r1")
                nc.vector.scalar_tensor_tensor(
                    out=mask2c, in0=pos2f, scalar=float(CAP), in1=m2sb[:, t, :],
                    op0=Alu.is_lt, op1=Alu.mult, accum_out=valid2,
                )
                junk2 = rt.tile([P, E], f32, tag="r8")
                nc.vector.scalar_tensor_tensor(
                    out=junk2, in0=pos2f, scalar=0.0, in1=m2sb[:, t, :],
                    op0=Alu.add, op1=Alu.mult, accum_out=p2_all[:, t:t + 1],
                )
                p2p1 = rt.tile([P, 1], f32, tag="r1")
                nc.vector.tensor_scalar(
                    p2p1, p2_all[:, t:t + 1], 1.0, None, op0=Alu.add
                )
                nc.vector.scalar_tensor_tensor(
                    out=p2e_sb[:, t, :], in0=m2sb[:, t, :], scalar=p2p1, in1=cneg1,
                    op0=Alu.mult, op1=Alu.add,
                )
                nc.vector.tensor_tensor(
                    g2f_all[:, t:t + 1], g2n_all[:, t:t + 1], valid2, op=Alu.mult
                )
                a6 = rt.tile([P, 1], f32, tag="r1")
                nc.vector.tensor_tensor(a6, p2_all[:, t:t + 1], valid2, op=Alu.mult)
                s2f = rt.tile([P, 1], f32, tag="r1")
                nc.vector.scalar_tensor_tensor(
                    out=s2f, in0=e2x512[:, t:t + 1], scalar=valid2, in1=a6,
                    op0=Alu.mult, op1=Alu.add,
                )
                nc.vector.tensor_copy(s2i[:, t:t + 1], s2f)

            # loss stats: col-sums of raw gates and pre-capacity mask_1
            st1 = psp.tile([1, E], f32, tag="ps")
            nc.tensor.matmul(st1, lhsT=ones_c, rhs=acc_raw, start=True, stop=True)
            nc.scalar.copy(stats_sb[:, 0:E], st1)
            st2 = psp.tile([1, E], f32, tag="ps")
            nc.tensor.matmul(st2, lhsT=ones_c, rhs=acc_m1, start=True, stop=True)
            nc.scalar.copy(stats_sb[:, E:2 * E], st2)
            nc.sync.dma_start(out=stats_d[:, :], in_=stats_sb)

            # ================ Phase 2: per-expert dispatch + FFN ================
            for e in range(E):
                # dispatch one-hots + eiT = x^T @ disp  -> [DIM, CAP]
                ei_ps = []
                for m in range(ND):
                    ei_ps.append(psp.tile([P, CAP], f32, tag="ps", name=f"eips{e}_{m}"))
                for t in range(NT):
                    disp = ph2.tile([P, CAP], f32r, tag="disp", bufs=3)
                    nc.vector.tensor_scalar(
                        disp, iota512f, p1e_sb[:, t, e:e + 1], None, op0=Alu.is_equal
                    )
                    nc.vector.scalar_tensor_tensor(
                        out=disp, in0=iota512f, scalar=p2e_sb[:, t, e:e + 1],
                        in1=disp, op0=Alu.is_equal, op1=Alu.add,
                    )
                    for m in range(ND):
                        nc.tensor.matmul(
                            ei_ps[m],
                            lhsT=x_sb[:, t, m * P:(m + 1) * P],
                            rhs=disp,
                            start=(t == 0), stop=(t == NT - 1),
                        )
                eiT_sb = ph2.tile([P, ND, CAP], f32r, tag="eit", bufs=1)
                for m in range(ND):
                    nc.vector.tensor_copy(eiT_sb[:, m, :], ei_ps[m])

                # FFN1 + gelu: hT[h, cap]
                hT_sb = ph2.tile([P, NH, CAP], f32r, tag="ht", bufs=1)
                for hm in range(NH):
                    w1s = ph2.tile([P, ND, P], f32r, tag="w1s", bufs=2)
                    nc.sync.dma_start(
                        out=w1s,
                        in_=w1_d[e, :, hm * P:(hm + 1) * P].rearrange(
                            "(kt p) h -> p kt h", p=P
                        ),
                    )
                    h_ps = psp.tile([P, CAP], f32, tag="ps", name=f"hps{e}_{hm}")
                    for km in range(ND):
                        nc.tensor.matmul(
                            h_ps, lhsT=w1s[:, km, :], rhs=eiT_sb[:, km, :],
                            start=(km == 0), stop=(km == ND - 1),
                        )
                    nc.scalar.activation(hT_sb[:, hm, :], h_ps, Act.Gelu)

                # FFN2: EO[cap, d] accumulated over h tiles
                eo_ps = []
                for j in range(NC_ * 2):
                    eo_ps.append(psp.tile([P, CAP], f32, tag="ps", name=f"eops{e}_{j}"))
                for kh in range(NH):
                    w2s = ph2.tile([P, DIM], f32r, tag="w2s", bufs=2)
                    nc.sync.dma_start(out=w2s, in_=w2_d[e, kh * P:(kh + 1) * P, :])
                    for cm in range(NC_):
                        for dn in range(2):
                            nc.tensor.matmul(
                                eo_ps[cm * 2 + dn],
                                lhsT=hT_sb[:, kh, cm * P:(cm + 1) * P],
                                rhs=w2s[:, dn * CAP:(dn + 1) * CAP],
                                start=(kh == 0), stop=(kh == NH - 1),
                            )
                eo_sb = ph2.tile([P, NC_ * 2, CAP], f32, tag="eo", bufs=1)
                for cm in range(NC_):
                    for dn in range(2):
                        nc.scalar.copy(eo_sb[:, cm * 2 + dn, :], eo_ps[cm * 2 + dn])
                        nc.sync.dma_start(
                            out=eo_d[
                                e * CAP + cm * P: e * CAP + (cm + 1) * P,
                                dn * CAP:(dn + 1) * CAP,
                            ],
                            in_=eo_sb[:, cm * 2 + dn, :],
                        )

            # ================ Phase 3: combine via row gathers ================
            import concourse.bass as bass_mod

            for t in range(NT):
                gA = ph3.tile([P, DIM], f32, tag="gA", bufs=2)
                nc.gpsimd.indirect_dma_start(
                    out=gA, out_offset=None, in_=eo_d[:, :],
                    in_offset=bass_mod.IndirectOffsetOnAxis(
                        ap=s1i[:, t:t + 1], axis=0
                    ),
                )
                gB = ph3.tile([P, DIM], f32, tag="gB", bufs=2)
                nc.gpsimd.indirect_dma_start(
                    out=gB, out_offset=None, in_=eo_d[:, :],
                    in_offset=bass_mod.IndirectOffsetOnAxis(
                        ap=s2i[:, t:t + 1], axis=0
                    ),
                )
                nc.vector.tensor_scalar_mul(gA, gA, g1f_all[:, t:t + 1])
                outt = ph3.tile([P, DIM], f32, tag="outt", bufs=2)
                nc.vector.scalar_tensor_tensor(
                    out=outt, in0=gB, scalar=g2f_all[:, t:t + 1], in1=gA,
                    op0=Alu.mult, op1=Alu.add,
                )
                nc.sync.dma_start(out=out_d[t * P:(t + 1) * P, :], in_=outt)

    nc.compile()
    return nc


def _get_nc():
    if "nc" not in _CACHE:
        _CACHE["nc"] = build_nc()
    return _CACHE["nc"]


def round_f32r(a):
    """Round fp32 to the fp32r grid (11 explicit mantissa bits, RNE)."""
    b = np.ascontiguousarray(a, dtype=np.float32).view(np.uint32)
    lsb = (b >> 12) & np.uint32(1)
    r = (b + np.uint32(0x7FF) + lsb) & np.uint32(0xFFFFF000)
    return r.view(np.float32)


def kernel(x, w_gating, w1, w2, rand_probs):
    from concourse.bass_utils import run_bass_kernel_spmd

    nc = _get_nc()
    x = np.ascontiguousarray(x, dtype=np.float32)
    rand_probs = np.ascontiguousarray(rand_probs, dtype=np.float32)
    w_gating = np.ascontiguousarray(w_gating, dtype=np.float32)
    w1r = round_f32r(w1)
    w2r = round_f32r(w2)
    xr = round_f32r(x)

    in_maps = [
        {"x": x[b], "xr": xr[b], "rand": rand_probs[b], "wg": w_gating,
         "w1": w1r, "w2": w2r}
        for b in range(B)
    ]
    res = run_bass_kernel_spmd(nc, in_maps, core_ids=list(range(B)))
    out = np.stack([res.results[b]["out"] for b in range(B)])
    stats = np.stack([res.results[b]["stats"][0] for b in range(B)])  # [B, 16]
    proxy_mean = stats[:, 0:E] / np.float32(N)
    density = stats[:, E:2 * E] / np.float32(N)
    loss = np.float32(np.mean(proxy_mean * density) * (E * E) * LOSS_COEF)
    return out, loss
